# revision 38
# baseline (speedup 1.0000x reference)
"""GRU cell (AnotherGRUCell) on 8 TRN2 NeuronCores.

Strategy: pure data-parallel over batch (8192 rows -> 1024 rows/core),
weights replicated. No collectives.

All on-chip compute is done in TRANSPOSED layout (units on the partition
axis, batch on the free axis):
  - matmul out[n, m] = sum_k W[k, n] * xT[k, m], with the weight tile as
    the stationary operand (lhsT) and xT/hT/rhT as the moving operand.
  - the r/u gate GEMMs x@Wi[:, :2u] + h@Wh[:, :2u] fuse into ONE 32-ktile
    PSUM accumulation over the concatenated operand [xT; hT].
  - the candidate GEMM x@Wi3 + (r*h)@Wh3 similarly accumulates over
    [xT; rhT]; rhT = sigmoid(gates) * hT is produced by ScalarE+VectorE
    already in the [k_part, m_free] layout the matmul needs -> zero
    on-chip transposes.
  - bias is per-partition in this layout, folded into the ScalarE
    activation (sigmoid/tanh) that reads PSUM directly.

The first gate pair is block-interleaved over the k loop so each
freshly-DMA'd x/h tile feeds 4 back-to-back matmuls (2 gate col-tiles x
2 batch chunks) into 4 PSUM banks, hiding the startup input-load
latency behind PE work; steady state interleaves the 2 batch chunks so
consecutive matmuls share the stationary weight tile.

Host side pre-transposes the x/h shards, packs weights into per-column-
tile slabs, casts to bf16, and transposes the [2048, 1024] per-core
output back to [1024, 2048].
"""

import numpy as np
import ml_dtypes

import concourse.bacc as bacc
import concourse.tile as tile
import concourse.mybir as mybir
from concourse.bass_utils import run_bass_kernel_spmd

N_CORES = 8
UNITS = 2048
IN_DIM = 2048
BATCH = 8192
B_LOC = BATCH // N_CORES  # 1024 batch rows per core

P = 128
KT_X = IN_DIM // P           # 16 k-tiles of x
KT_H = UNITS // P            # 16 k-tiles of h
KT = KT_X + KT_H             # 32 contraction k-tiles for [x; h]
NT_G = (2 * UNITS) // P      # 32 gate col-tiles (r: 0..15, u: 16..31)
NT_C = UNITS // P            # 16 candidate col-tiles
M_CHUNK = 512
MC = B_LOC // M_CHUNK        # 2 moving chunks per core

BF16 = mybir.dt.bfloat16
F32 = mybir.dt.float32
NP_BF16 = ml_dtypes.bfloat16

_CACHED_NC = None

# test.py sets TRACE=True to capture the NTFF profile (exec_time_ns +
# perfetto trace); the graded path leaves it off. LAST_RESULTS holds the
# BassKernelResults of the most recent run.
TRACE = False
LAST_RESULTS = None


def _build():
    nc = bacc.Bacc("TRN2", target_bir_lowering=False, debug=False)

    xT = nc.dram_tensor("xT", [KT_X, P, B_LOC], BF16, kind="ExternalInput")
    hT = nc.dram_tensor("hT", [KT_H, P, B_LOC], BF16, kind="ExternalInput")
    w_g = nc.dram_tensor("w_g", [NT_G, P, KT * P], BF16, kind="ExternalInput")
    w_c = nc.dram_tensor("w_c", [NT_C, P, KT * P], BF16, kind="ExternalInput")
    # biases transposed: one [128, n_tiles] tensor per gate set -> 1 DMA each
    b_g = nc.dram_tensor("b_g", [P, NT_G], F32, kind="ExternalInput")
    b_c = nc.dram_tensor("b_c", [P, NT_C], F32, kind="ExternalInput")
    out = nc.dram_tensor("out", [NT_C, P, B_LOC], F32, kind="ExternalOutput")

    SIG = mybir.ActivationFunctionType.Sigmoid
    TANH = mybir.ActivationFunctionType.Tanh

    with tile.TileContext(nc) as tc:
        with (
            tc.tile_pool(name="resident", bufs=1) as res,
            tc.tile_pool(name="wslab", bufs=4) as wp,
            tc.tile_pool(name="psum", bufs=8, space="PSUM") as pp,
            tc.tile_pool(name="stage", bufs=2) as sp,
            tc.tile_pool(name="bias", bufs=1) as bp,
        ):
            x_tiles = [
                res.tile([P, B_LOC], BF16, tag=f"x{j}", name=f"x{j}")
                for j in range(KT_X)
            ]
            h_tiles = [
                res.tile([P, B_LOC], BF16, tag=f"h{j}", name=f"h{j}")
                for j in range(KT_H)
            ]
            rh_tiles = [
                res.tile([P, B_LOC], BF16, tag=f"rh{j}", name=f"rh{j}")
                for j in range(KT_H)
            ]
            u_tiles = [
                res.tile([P, B_LOC], BF16, tag=f"u{j}", name=f"u{j}")
                for j in range(NT_C)
            ]

            # Startup DMAs in exact consumption order of the first gate
            # pair, interleaved across both HWDGE rings. The rings drain
            # roughly FIFO at HBM rate, so block jb's operands (weight
            # chunk jb/8 of both slabs + src tiles jb..jb+7) are emitted
            # right before the block needs them.
            # Graduated chunk sizes: a tiny first chunk so the very first
            # matmul's dependencies are only ~0.6MB of DMA.
            CHUNKS = [2, 6, 8, 8, 8]
            CB = [0, 2, 8, 16, 24, 32]  # chunk k-tile boundaries
            ws_first = [[None] * len(CHUNKS) for _ in range(2)]  # [t][chunk]
            src_dma = {}  # j -> (engine, dst tile, src ap)
            for j in range(KT_X):
                eng = nc.sync if j % 2 == 0 else nc.scalar
                src_dma[j] = (eng, x_tiles[j], xT[j, :, :])
            for j in range(KT_H):
                eng = nc.scalar if j % 2 == 0 else nc.sync
                src_dma[KT_X + j] = (eng, h_tiles[j], hT[j, :, :])
            for c, cw in enumerate(CHUNKS):
                for t in range(2):
                    w = wp.tile(
                        [P, cw * P], BF16, tag=f"wg{t}_{c}", name=f"wg{t}_{c}",
                        bufs=1,
                    )
                    (nc.sync if t == 0 else nc.scalar).dma_start(
                        w[:], w_g[t, :, CB[c] * P:CB[c + 1] * P]
                    )
                    ws_first[t][c] = w
                for j in range(CB[c], CB[c + 1]):
                    eng, dst, src = src_dma[j]
                    eng.dma_start(dst[:], src)

            bg_all = bp.tile([P, NT_G], F32, tag="bg", name="bg_all")
            nc.sync.dma_start(bg_all[:], b_g[:, :])
            bc_all = bp.tile([P, NT_C], F32, tag="bc", name="bc_all")
            nc.scalar.dma_start(bc_all[:], b_c[:, :])

            gate_src = x_tiles + h_tiles
            cand_src = x_tiles + rh_tiles

            def act_sig(t, m, ps):
                """sigmoid(psum + b) -> rh (r gates, premultiplied by h) or u."""
                ms = slice(m * M_CHUNK, (m + 1) * M_CHUNK)
                if t < NT_C:
                    rt = sp.tile([P, M_CHUNK], BF16, tag="rtmp", name=f"r{t}_{m}")
                    nc.scalar.activation(rt[:], ps[:], SIG, bias=bg_all[:, t:t + 1])
                    nc.vector.tensor_mul(rh_tiles[t][:, ms], rt[:], h_tiles[t][:, ms])
                else:
                    nc.scalar.activation(
                        u_tiles[t - NT_C][:, ms], ps[:], SIG,
                        bias=bg_all[:, t:t + 1],
                    )

            # Phase 1: gates; t 0..15 -> r, 16..31 -> u.
            #
            # The FIRST pair of gate tiles is block-interleaved (4 psum
            # groups, one block per weight chunk) so the PE has 4 matmuls
            # to run per freshly-arrived x/h tile during the startup
            # loads instead of stalling on the whole operand set.
            t0_groups = [(0, 0), (0, 1), (1, 0), (1, 1)]
            pss0 = [
                pp.tile([P, M_CHUNK], F32, tag="psum", name=f"psg0_{i}")
                for i in range(4)
            ]
            for c in range(len(CHUNKS)):
                for i, (t, m) in enumerate(t0_groups):
                    ms = slice(m * M_CHUNK, (m + 1) * M_CHUNK)
                    for j in range(CB[c], CB[c + 1]):
                        wch = ws_first[t][c]
                        jj = j - CB[c]
                        nc.tensor.matmul(
                            pss0[i][:],
                            wch[:, jj * P:(jj + 1) * P],
                            gate_src[j][:, ms],
                            start=(j == 0),
                            stop=(j == KT - 1),
                        )
            for i, (t, m) in enumerate(t0_groups):
                act_sig(t, m, pss0[i])

            # Weight slabs are loaded in PAIRS (two gate col-tiles per DMA):
            # the PE pays one skipped 216ns beat at each new weight tile's
            # first-use semaphore wait, so halving the tile count halves
            # that cost. Within each t the k loop is m-interleaved so
            # consecutive matmuls share the stationary weight tile.
            def slab_pair(w_dram, tp, name):
                ws = wp.tile([P, 2 * KT * P], BF16, tag="wslab", name=name, bufs=2)
                nc.sync.dma_start(ws[:, :KT * P], w_dram[tp, :, :])
                nc.scalar.dma_start(ws[:, KT * P:], w_dram[tp + 1, :, :])
                return ws

            def gemm_group(ws, ti, src_tiles, t, act_fn):
                """One m-interleaved accumulation pair for gate col-tile t,
                using half `ti` of the pair slab `ws`."""
                psl = [
                    pp.tile([P, M_CHUNK], F32, tag="psum", name=f"ps{t}_{m}")
                    for m in range(MC)
                ]
                for j in range(KT):
                    off = (ti * KT + j) * P
                    for m in range(MC):
                        ms = slice(m * M_CHUNK, (m + 1) * M_CHUNK)
                        nc.tensor.matmul(
                            psl[m][:],
                            ws[:, off:off + P],
                            src_tiles[j][:, ms],
                            start=(j == 0),
                            stop=(j == KT - 1),
                        )
                for m in range(MC):
                    act_fn(t, m, psl[m])

            for tp in range(2, NT_G, 2):
                ws = slab_pair(w_g, tp, f"wg{tp}")
                for ti, t in enumerate((tp, tp + 1)):
                    gemm_group(ws, ti, gate_src, t, act_sig)

            # Phase 2: candidate GEMM + tanh + output combine
            # h_t = u * (h - cand) + cand
            def cand_epilogue(t, m, mw, ps):
                ms = slice(m * mw, (m + 1) * mw)
                cand = sp.tile([P, mw], F32, tag="cand", name=f"c{t}_{m}")
                nc.scalar.activation(cand[:], ps[:], TANH, bias=bc_all[:, t:t + 1])
                d = sp.tile([P, mw], F32, tag="d", name=f"d{t}_{m}")
                nc.vector.tensor_sub(d[:], h_tiles[t][:, ms], cand[:])
                d2 = sp.tile([P, mw], F32, tag="d2", name=f"d2{t}_{m}")
                nc.vector.tensor_mul(d2[:], u_tiles[t][:, ms], d[:])
                ht = sp.tile([P, mw], F32, tag="ht", name=f"ht{t}_{m}")
                nc.vector.tensor_add(ht[:], d2[:], cand[:])
                nc.sync.dma_start(out[t, :, ms], ht[:])

            def cand_group(ws, ti, t):
                gemm_group(
                    ws, ti, cand_src, t,
                    lambda t, m, ps: cand_epilogue(t, m, M_CHUNK, ps),
                )

            def cand_group_tapered(ws, ti, t):
                # Last tile: narrow sequential chunks so epilogues stagger
                # and the post-final-matmul tail stays short.
                mw = M_CHUNK // 2
                for m in range(B_LOC // mw):
                    ms = slice(m * mw, (m + 1) * mw)
                    ps = pp.tile([P, mw], F32, tag="psum", name=f"psc{t}_{m}")
                    for j in range(KT):
                        off = (ti * KT + j) * P
                        nc.tensor.matmul(
                            ps[:],
                            ws[:, off:off + P],
                            cand_src[j][:, ms],
                            start=(j == 0),
                            stop=(j == KT - 1),
                        )
                    cand_epilogue(t, m, mw, ps)

            for tp in range(0, NT_C, 2):
                ws = slab_pair(w_c, tp, f"wc{tp}")
                for ti, t in enumerate((tp, tp + 1)):
                    if t == NT_C - 1:
                        cand_group_tapered(ws, ti, t)
                    else:
                        cand_group(ws, ti, t)

    nc.compile()
    return nc


def _get_nc():
    global _CACHED_NC
    if _CACHED_NC is None:
        _CACHED_NC = _build()
    return _CACHED_NC


def _pack_w(w):
    """[K, N] fp32 -> [N/128, 128, K] bf16 slab layout:
    slab[t, p, j*128 + c] = w[j*128 + p, t*128 + c]"""
    K, N = w.shape
    a = w.reshape(K // P, P, N // P, P).transpose(2, 1, 0, 3)
    return np.ascontiguousarray(a).astype(NP_BF16).reshape(N // P, P, K)


def kernel(x_t, h_tm1, input_weight, hidden_state_weight, bias):
    x_t = np.asarray(x_t, dtype=np.float32)
    h_tm1 = np.asarray(h_tm1, dtype=np.float32)
    input_weight = np.asarray(input_weight, dtype=np.float32)
    hidden_state_weight = np.asarray(hidden_state_weight, dtype=np.float32)
    bias = np.asarray(bias, dtype=np.float32)

    u = UNITS
    # Gate weights: [x; h] @ [Wi[:, :2u]; Wh[:, :2u]]
    w_gate = np.concatenate(
        [input_weight[:, : 2 * u], hidden_state_weight[:, : 2 * u]], axis=0
    )  # [4096, 4096]
    w_cand = np.concatenate(
        [input_weight[:, 2 * u:], hidden_state_weight[:, 2 * u:]], axis=0
    )  # [4096, 2048]
    w_g_np = _pack_w(w_gate)  # [32, 128, 4096] bf16
    w_c_np = _pack_w(w_cand)  # [16, 128, 4096] bf16
    b_g_np = np.ascontiguousarray(bias[: 2 * u].reshape(NT_G, P).T, dtype=np.float32)
    b_c_np = np.ascontiguousarray(bias[2 * u:].reshape(NT_C, P).T, dtype=np.float32)

    in_maps = []
    for i in range(N_CORES):
        sl = slice(i * B_LOC, (i + 1) * B_LOC)
        xT_np = x_t[sl].T.astype(NP_BF16).reshape(KT_X, P, B_LOC)
        hT_np = h_tm1[sl].T.astype(NP_BF16).reshape(KT_H, P, B_LOC)
        in_maps.append(
            {
                "xT": np.ascontiguousarray(xT_np),
                "hT": np.ascontiguousarray(hT_np),
                "w_g": w_g_np,
                "w_c": w_c_np,
                "b_g": b_g_np,
                "b_c": b_c_np,
            }
        )

    nc = _get_nc()
    res = run_bass_kernel_spmd(
        nc, in_maps, core_ids=list(range(N_CORES)), trace=TRACE
    )
    global LAST_RESULTS
    LAST_RESULTS = res

    h_t = np.empty((BATCH, UNITS), dtype=np.float32)
    for i in range(N_CORES):
        o = np.asarray(res.results[i]["out"], dtype=np.float32)
        h_t[i * B_LOC:(i + 1) * B_LOC] = o.reshape(UNITS, B_LOC).T
    return h_t
